# revision 8
# baseline (speedup 1.0000x reference)
"""Trainium2 Bass kernel for cached multi-head self-attention decode step.

Problem (hardcoded):
  B=16, T=8, C=1024, n_head=16, head_dim=64, Lcache=4096, layer index 1.
  reference:
    q = x@Wq.T + bq ; key = x@Wk.T ; value = x@Wv.T + bv
    K = concat(kv_cache[:,1,0], key) ; V = concat(kv_cache[:,1,1], value)
    out = softmax((q*s)(K*s)^T) @ V @ Wo.T + bo      (s = hd**-0.25)
    returns (out, key, value)

Sharding: data-parallel over batch. 8 cores x 2 batches each. No collectives.

v3: fused streaming attention.
  - weights + x bf16; KV cache and q packing fp8e4m3 (PE moving-operand
    ingest is 128 B/cycle, so fp8 streams 2x bf16; DMA also halves).
  - scores in fp8 DoubleRow (2 chunk k-tiles per pass = 256 B/cycle);
    output halves g sit in [64, 512] PSUM tiles at tile position (0,0),
    an ISA requirement for DoubleRow.
  - softmax scale folded into the exp activation, biased by -ln(8) so
    exp values fit fp8/bf16 range (cancels in deferred normalization).
  - per (window, batch): scores -> exp into W window [64, (g, 512)] ->
    ONE DMA-crossbar transpose -> wts [128, (g, chunk), 64] -> S@V
    matmuls with bf16 stationary x fp8 moving V (mixed dtypes verified
    exact on HW). The PE runs no W transposes, the DVE does no W copies,
    and KT/V DMAs interleave across the whole loop with no phase
    barriers.
  - all Wqkv+WoT prefetched up front; out projection reads On via a
    second crossbar transpose.
"""

import sys
import types

import numpy as np
import ml_dtypes

# ---- hardcoded problem geometry ----
B, T, C = 16, 8, 1024
H, HD = 16, 64
L = 4096            # cached length
LT = L + T          # total keys
NCORES = 8
BPC = B // NCORES   # batches per core = 2
M = BPC * T         # queries per core = 16
P = 128
CH = C // P         # 8 c-chunks
NW = L // 512       # 8 score windows of 512
SCALE = float(HD) ** -0.5  # folded into the exp activation
BEXP = -2.0794415416798357  # -ln(8)

_CACHE = {}


def _ensure_ntff_hook():
    """run_bass_kernel_spmd(trace=True) under axon needs antenv.axon_hooks;
    shim it from the boot module if the image's antenv lacks it."""
    try:
        import antenv.axon_hooks  # noqa: F401
        return
    except ImportError:
        pass
    try:
        import trn_agent_boot.trn_boot as tb
        hook = tb._ntff_profile_via_ctypes("/opt/axon/libaxon_pjrt.so")
    except Exception:
        hook = None
    mod = types.ModuleType("antenv.axon_hooks")
    mod.get_axon_ntff_profile_hook = lambda: hook
    mod.set_axon_ntff_profile_hook = lambda h: None
    sys.modules["antenv.axon_hooks"] = mod


def _build():
    import concourse.bacc as bacc
    import concourse.mybir as mybir
    import concourse.tile as tile
    from concourse.masks import make_identity

    f32 = mybir.dt.float32
    bf16 = mybir.dt.bfloat16
    f8 = mybir.dt.float8e4
    DR = mybir.MatmulPerfMode.DoubleRow

    nc = bacc.Bacc(None, target_bir_lowering=False)

    # ---- dram I/O ----
    xTr = nc.dram_tensor("xTr", [P, CH, M], bf16, kind="ExternalInput")
    KT = nc.dram_tensor("KT", [BPC, CH, P, L], f8, kind="ExternalInput")
    Vd = nc.dram_tensor("Vd", [BPC, L // P, P, C], f8, kind="ExternalInput")
    Wqkv = nc.dram_tensor("Wqkv", [CH, P, 3 * C], bf16, kind="ExternalInput")
    WoT = nc.dram_tensor("WoT", [CH, P, C], bf16, kind="ExternalInput")
    bqs = nc.dram_tensor("bqs", [P, CH], f32, kind="ExternalInput")
    bvb = nc.dram_tensor("bvb", [M, C], f32, kind="ExternalInput")
    bob = nc.dram_tensor("bob", [M, C], f32, kind="ExternalInput")
    out_d = nc.dram_tensor("out", [M, C], f32, kind="ExternalOutput")
    key_d = nc.dram_tensor("key", [M, C], f32, kind="ExternalOutput")
    val_d = nc.dram_tensor("value", [M, C], f32, kind="ExternalOutput")

    AF = mybir.ActivationFunctionType
    AX = mybir.AxisListType
    OP = mybir.AluOpType

    with tile.TileContext(nc) as tc:
        with (
            tc.tile_pool(name="const", bufs=1) as cpool,
            tc.tile_pool(name="wqkv", bufs=1) as wpool,
            tc.tile_pool(name="wo", bufs=1) as wopool,
            tc.tile_pool(name="kt", bufs=3) as ktpool,
            tc.tile_pool(name="v", bufs=4) as vpool,
            tc.tile_pool(name="wwin", bufs=3) as wwpool,
            tc.tile_pool(name="wts", bufs=4) as wtpool,
            tc.tile_pool(name="big", bufs=1) as big,
            tc.tile_pool(name="ps", bufs=1, space="PSUM") as pp,
        ):
            # ---- constants / small inputs (sync queue) ----
            ident = cpool.tile([P, P], f32, tag="ident", name="ident")
            make_identity(nc, ident)
            ident_bf = cpool.tile([P, P], bf16, tag="ident_bf", name="ident_bf")
            nc.vector.tensor_copy(out=ident_bf[:], in_=ident[:])
            xT_sb = cpool.tile([P, CH, M], bf16, tag="xT", name="xT")
            nc.sync.dma_start(xT_sb[:], xTr[:])
            bqs_sb = cpool.tile([P, CH], f32, tag="bqs", name="bqs")
            nc.sync.dma_start(bqs_sb[:], bqs[:])
            bvb_sb = cpool.tile([M, C], f32, tag="bvb", name="bvb")
            nc.sync.dma_start(bvb_sb[:], bvb[:])
            bob_sb = cpool.tile([M, C], f32, tag="bob", name="bob")
            nc.sync.dma_start(bob_sb[:], bob[:])
            bexp = cpool.tile([P, 1], f32, tag="bexp", name="bexp")
            nc.gpsimd.memset(bexp[:], BEXP)

            # ---- weight prefetch (scalar queue): all Wqkv then all WoT ----
            wq_t = []
            for ci in range(CH):
                wt = wpool.tile([P, 3 * C], bf16, tag=f"wqkv{ci}",
                                name=f"wqkv{ci}")
                nc.scalar.dma_start(wt[:], Wqkv[ci])
                wq_t.append(wt)
            wo_t = []
            for ci in range(CH):
                wo = wopool.tile([P, C], bf16, tag=f"wo{ci}", name=f"wo{ci}")
                nc.scalar.dma_start(wo[:], WoT[ci])
                wo_t.append(wo)

            # ---- projections q/k/v (bf16 x bf16 -> f32 psum) ----
            ps_proj = [pp.tile([M, 512], f32, tag=t, name=f"proj_{t}")
                       for t in ("o0", "o1", "o2", "o3", "sc00", "sc01")]
            for ci in range(CH):
                for j, ps in enumerate(ps_proj):
                    nc.tensor.matmul(
                        ps[:], xT_sb[:, ci, :],
                        wq_t[ci][:, j * 512:(j + 1) * 512],
                        start=(ci == 0), stop=(ci == CH - 1),
                    )

            q_nat = big.tile([P, C], f32, tag="q_nat", name="q_nat")
            k_nat = big.tile([P, C], f32, tag="k_nat", name="k_nat")
            v_nat = big.tile([P, C], f32, tag="v_nat", name="v_nat")
            for t in (q_nat, k_nat, v_nat):
                nc.gpsimd.memset(t[:], 0.0)
            for j in range(2):
                sl = slice(j * 512, (j + 1) * 512)
                nc.scalar.copy(q_nat[0:M, sl], ps_proj[0 + j][:])
                nc.scalar.copy(k_nat[0:M, sl], ps_proj[2 + j][:])
                nc.scalar.copy(v_nat[0:M, sl], ps_proj[4 + j][:])
            nc.vector.tensor_add(out=v_nat[0:M, :], in0=v_nat[0:M, :], in1=bvb_sb[:])
            nc.sync.dma_start(key_d[:], k_nat[0:M, :])
            nc.sync.dma_start(val_d[:], v_nat[0:M, :])

            # ---- DoubleRow q packing + kT via PE transpose ----
            # Qdr[b][g][p] is lhsT [128, 2, 64]: pair p of half g covers
            # chunks (4g+2p, 4g+2p+1) = heads 8g+4p .. 8g+4p+3. Column
            # j = 8*h_local + t. Block for head h sits at rows 64*(h%2),
            # ktile (h//2) - (4g+2p).
            Qdr = big.tile([P, BPC, 2, 2, 2, 64], f8, tag="Qdr", name="Qdr")
            nc.gpsimd.memset(Qdr[:], 0.0)
            kT = big.tile([P, CH, M], f8, tag="kT", name="kT")
            for ci in range(CH):
                sl = slice(ci * P, (ci + 1) * P)
                tp = pp.tile([P, P], f32, tag="sc10", name="tpq")
                nc.tensor.transpose(tp[:], q_nat[:, sl], ident)
                g = ci // 4
                p_ = (ci % 4) // 2
                for b in range(BPC):
                    for r in range(2):
                        h_local = 2 * ci + r - 8 * g
                        u = h_local - 4 * p_
                        nc.scalar.activation(
                            Qdr[64 * r:64 * r + 64, b, g, p_, u // 2,
                                8 * h_local:8 * h_local + 8],
                            tp[64 * r:64 * r + 64, b * T:(b + 1) * T],
                            AF.Identity, bias=bqs_sb[64 * r:64 * r + 64, ci:ci + 1],
                        )
                tp2 = pp.tile([P, P], f32, tag="sc11", name="tpk")
                nc.tensor.transpose(tp2[:], k_nat[:, sl], ident)
                nc.vector.tensor_copy(out=kT[:, ci, :], in_=tp2[:, 0:M])

            # vpad: new-kv V rows for the S@V tail (both batches)
            vpad = big.tile([P, C], f8, tag="vpad", name="vpad")
            nc.gpsimd.memset(vpad[:], 0.0)
            nc.vector.tensor_copy(out=vpad[0:M, :], in_=v_nat[0:M, :])

            # ---- fused attention stream ----
            sums, rsum, On = {}, {}, {}
            ops_b = {}
            for b in range(BPC):
                for g in range(2):
                    sums[b, g] = big.tile([64, 16], f32, tag=f"sums{b}{g}",
                                          name=f"sums{b}{g}")
                ops_b[b] = [pp.tile([P, 512], f32, tag=f"o{2 * b + j}",
                                    name=f"o{2 * b + j}") for j in range(2)]

            # software pipeline: scores/exp/transpose for iteration `it`
            # run ahead; the S@V matmuls for `it` are issued two iterations
            # later so the xbar-transpose latency never stalls the in-order
            # PE queue.
            DEPTH = 2
            pend = {}

            def issue_sv(it):
                b_, (wts_, vt_) = it % 2, pend.pop(it)
                for tt in range(4):
                    for j in range(2):
                        nc.tensor.matmul(
                            ops_b[b_][j][:], wts_[:, tt, :, :],
                            vt_[:, tt, j * 512:(j + 1) * 512],
                            start=(it // 2 == 0 and tt == 0), stop=False,
                        )

            for lw in range(NW):
                for b in range(BPC):
                    it = 2 * lw + b
                    kt = ktpool.tile([P, CH, 512], f8, tag="kt", name="kt")
                    nc.sync.dma_start(
                        kt[:],
                        KT[b].rearrange("ch pi l -> pi ch l")[
                            :, :, lw * 512:(lw + 1) * 512],
                    )
                    vt = vpool.tile([P, 4, C], f8, tag="v", name="v")
                    nc.scalar.dma_start(
                        vt[:],
                        Vd[b].rearrange("t pi c -> pi t c")[
                            :, lw * 4:(lw + 1) * 4, :],
                    )
                    # W window [64 hq-half, (chunk tt, g, l128)]: one xbar
                    # transpose covers both halves and all 4 chunks, and
                    # chunk-major order keeps each chunk's (g, 64) block
                    # contiguous in wts so the S@V lhsT has one free dim.
                    wwin = wwpool.tile([64, 4, 2, P], bf16, tag="ww", name="ww")
                    par = it % 2
                    for g in range(2):
                        sp = pp.tile([64, 512], f32, tag=f"sc{par}{g}",
                                     name=f"sp{par}{g}")
                        for p_ in range(2):
                            nc.tensor.matmul(
                                sp[:],
                                Qdr[:, b, g, p_, :, :],
                                kt[:, 4 * g + 2 * p_:4 * g + 2 * p_ + 2, :],
                                start=(p_ == 0), stop=(p_ == 1),
                                perf_mode=DR,
                            )
                        nc.scalar.activation(
                            wwin[:, :, g, :], sp[:],
                            AF.Exp, scale=SCALE, bias=bexp[0:64],
                            accum_out=sums[b, g][:, lw:lw + 1],
                        )
                    # wts[p, tt, g, j] = wwin[j, tt, g, p]
                    wts = wtpool.tile([P, 4, 2, 64], bf16, tag="wt", name="wt")
                    nc.sync.dma_start_transpose(wts[:], wwin[:])
                    pend[it] = (wts, vt)
                    if it >= DEPTH:
                        issue_sv(it - DEPTH)
            for it in sorted(pend):
                issue_sv(it)

            # ---- tail: new keys, sums, normalization ----
            wt32 = {}
            for b in range(BPC):
                wt32[b] = big.tile([P, P], bf16, tag=f"wt32_{b}",
                                   name=f"wt32_{b}")
                nc.gpsimd.memset(wt32[b][:], 0.0)
                for g in range(2):
                    spn = pp.tile([64, 512], f32, tag=f"sc{b % 2}{g}",
                                  name=f"spn{g}")
                    for p_ in range(2):
                        nc.tensor.matmul(
                            spn[:, 0:T],
                            Qdr[:, b, g, p_, :, :],
                            kT[:, 4 * g + 2 * p_:4 * g + 2 * p_ + 2,
                               b * T:(b + 1) * T],
                            start=(p_ == 0), stop=(p_ == 1),
                            perf_mode=DR,
                        )
                    wn = big.tile([64, M], bf16, tag=f"wn{b}{g}",
                                  name=f"wn{b}{g}")
                    nc.gpsimd.memset(wn[:], 0.0)
                    nc.scalar.activation(
                        wn[:, b * T:(b + 1) * T], spn[:, 0:T],
                        AF.Exp, scale=SCALE, bias=bexp[0:64],
                        accum_out=sums[b, g][:, NW:NW + 1],
                    )
                    rs = big.tile([64, 1], f32, tag=f"rs{b}{g}",
                                  name=f"rs{b}{g}")
                    nc.vector.tensor_reduce(
                        out=rs[:], in_=sums[b, g][:, 0:NW + 1], axis=AX.X,
                        op=OP.add)
                    rsum[b, g] = big.tile([64, 1], f32, tag=f"rsum{b}{g}",
                                          name=f"rsum{b}{g}")
                    nc.vector.reciprocal(rsum[b, g][:], rs[:])
                    # W_new^T: transpose [64, M] -> [M, 64]; batch-b rows
                    # land at b*T via the shifted input columns
                    tpn = pp.tile([P, P], bf16, tag=f"sc{b % 2}{g}",
                                  name="tpn")
                    nc.tensor.transpose(
                        tpn[0:M, 0:64], wn[:], ident_bf[0:64, 0:64])
                    nc.vector.tensor_copy(
                        out=wt32[b][0:M, 64 * g:64 * g + 64],
                        in_=tpn[0:M, 0:64])
            for b in range(BPC):
                for j in range(2):
                    nc.tensor.matmul(
                        ops_b[b][j][:], wt32[b][:],
                        vpad[:, j * 512:(j + 1) * 512],
                        start=False, stop=True,
                    )
                # normalize rows while copying out of PSUM, per hq half
                On[b] = big.tile([P, C], bf16, tag=f"On{b}", name=f"On{b}")
                for j in range(2):
                    for g in range(2):
                        nc.scalar.activation(
                            On[b][64 * g:64 * g + 64, j * 512:(j + 1) * 512],
                            ops_b[b][j][64 * g:64 * g + 64, :], AF.Copy,
                            scale=rsum[b, g][:],
                        )

            # ---- wvT via xbar transpose of On ----
            # OnT[c_local, ci, row]; head of c_local<64 is 2ci else 2ci+1:
            # two 64-partition block copies per (b, ci).
            wvT = big.tile([P, CH, M], bf16, tag="wvT", name="wvT")
            for b in range(BPC):
                OnT = big.tile([P, CH, P], bf16, tag=f"OnT{b}", name=f"OnT{b}")
                nc.scalar.dma_start_transpose(OnT[:], On[b][:])
                for ci in range(CH):
                    nc.vector.tensor_copy(
                        out=wvT[0:64, ci, b * T:(b + 1) * T],
                        in_=OnT[0:64, ci, 16 * ci:16 * ci + 8])
                    nc.vector.tensor_copy(
                        out=wvT[64:P, ci, b * T:(b + 1) * T],
                        in_=OnT[64:P, ci, 16 * ci + 8:16 * ci + 16])

            ps_fin = [pp.tile([M, 512], f32, tag=f"sc{j}0", name=f"fin{j}")
                      for j in range(2)]
            for ci in range(CH):
                for j in range(2):
                    nc.tensor.matmul(
                        ps_fin[j][:], wvT[:, ci, :],
                        wo_t[ci][:, j * 512:(j + 1) * 512],
                        start=(ci == 0), stop=(ci == CH - 1),
                    )
            fin = big.tile([M, C], f32, tag="fin", name="fin")
            for j in range(2):
                nc.scalar.copy(fin[:, j * 512:(j + 1) * 512], ps_fin[j][:])
            nc.vector.tensor_add(out=fin[:], in0=fin[:], in1=bob_sb[:])
            nc.sync.dma_start(out_d[:], fin[:])

    nc.compile()
    return nc


def _prep_host(x, kv_cache, Wq, bq, Wk, Wv, bv, Wo, bo):
    f8 = ml_dtypes.float8_e4m3fn
    bf = ml_dtypes.bfloat16
    f32 = np.float32
    x = np.asarray(x, f32)
    kv = np.asarray(kv_cache)
    Wq = np.asarray(Wq, f32); bq = np.asarray(bq, f32)
    Wk = np.asarray(Wk, f32); Wv = np.asarray(Wv, f32); bv = np.asarray(bv, f32)
    Wo = np.asarray(Wo, f32); bo = np.asarray(bo, f32)

    # K-cache transposed on host -> all device loads contiguous
    KT_all = np.ascontiguousarray(
        np.asarray(kv[:, 1, 0], f32).transpose(0, 2, 1)).astype(f8)  # [B, C, L]
    V_all = np.ascontiguousarray(np.asarray(kv[:, 1, 1], f32)).astype(f8)

    Wqkv = np.ascontiguousarray(
        np.concatenate([Wq.T, Wk.T, Wv.T], axis=1)).astype(bf).reshape(CH, P, 3 * C)
    WoT8 = np.ascontiguousarray(Wo.T).astype(bf).reshape(CH, P, C)
    bqs = np.ascontiguousarray(bq.reshape(CH, P).T)  # [P, CH], natural scale
    bvb = np.ascontiguousarray(np.tile(bv, (M, 1)))
    bob = np.ascontiguousarray(np.tile(bo, (M, 1)))

    in_maps = []
    for c in range(NCORES):
        xc = x[c * BPC:(c + 1) * BPC].reshape(M, C)
        xTr = np.ascontiguousarray(
            xc.reshape(M, CH, P).transpose(2, 1, 0)).astype(bf)
        in_maps.append({
            "xTr": xTr,
            "KT": np.ascontiguousarray(
                KT_all[c * BPC:(c + 1) * BPC]).reshape(BPC, CH, P, L),
            "Vd": np.ascontiguousarray(
                V_all[c * BPC:(c + 1) * BPC]).reshape(BPC, L // P, P, C),
            "Wqkv": Wqkv, "WoT": WoT8,
            "bqs": bqs, "bvb": bvb, "bob": bob,
        })
    return in_maps


def kernel(x, kv_cache, Wq, bq, Wk, Wv, bv, Wo, bo, _trace=False, _tmpdir=None):
    from concourse.bass_utils import run_bass_kernel_spmd

    _ensure_ntff_hook()
    if "nc" not in _CACHE:
        _CACHE["nc"] = _build()
    nc = _CACHE["nc"]

    in_maps = _prep_host(x, kv_cache, Wq, bq, Wk, Wv, bv, Wo, bo)
    res = run_bass_kernel_spmd(
        nc, in_maps, core_ids=list(range(NCORES)),
        trace=_trace, tmpdir=_tmpdir,
    )
    out = np.empty((B, T, C), np.float32)
    key_o = np.empty((B, T, C), np.float32)
    val_o = np.empty((B, T, C), np.float32)
    for c in range(NCORES):
        r = res.results[c]
        sl = slice(c * BPC, (c + 1) * BPC)
        out[sl] = r["out"].reshape(BPC, T, C)
        key_o[sl] = r["key"].reshape(BPC, T, C)
        val_o[sl] = r["value"].reshape(BPC, T, C)
    kernel._last_exec_time_ns = res.exec_time_ns
    kernel._last_results = res
    return (out, key_o, val_o)
